# revision 2
# baseline (speedup 1.0000x reference)
"""Conv2d 3x3 VALID via 1D Winograd F(2,3) along H, batch-sharded on 8 cores.

Problem: input [32,128,64,64] f32, weights [256,128,3,3] f32 ->
output [32,256,62,62] f32 (stride 1, no padding).

Host-side (per kernel() call):
  - image cast f32 -> bf16, then the Winograd input transform along H:
    for row-pair p (31 pairs = 62 out rows), with d_i = image row 2p+i:
      V0 = d0 - d2, V1 = d1 + d2, V2 = d2 - d1, V3 = d1 - d3   (bf16)
  - weight transform G w over kh per kw and Cout half:
      W'0 = w0, W'1 = (w0+w1+w2)/2, W'2 = (w0-w1+w2)/2, W'3 = w2
    shipped pre-transposed as lhsT [ci, (k kw h co)] bf16.

Device-side (per core, 4 images):
  - M_k[co, p, x] = sum_kw sum_ci V_k[ci, p, x+kw] * W'_k,kw[ci, co]
    accumulated over kw into PSUM bank k (3 matmuls, N = npairs*62 <= 496).
  - per block of <=8 row pairs and Cout half: 12 matmuls into 4 banks,
    then the inverse transform:
      even out rows = M0 + M1 + M2   (DVE tensor_reduce over the 3 banks)
      odd  out rows = M1 - M2 - M3   (DVE sub + ACT negate-copy + GPSIMD add)
  - staging tile [co, 2*npairs, 62] f32 -> DMA to DRAM.
"""

import numpy as np
import ml_dtypes

import concourse.bass as bass
import concourse.mybir as mybir
import concourse.tile as tile
from concourse import bacc
from concourse.bass_utils import run_bass_kernel_spmd

F32 = mybir.dt.float32
BF16 = mybir.dt.bfloat16

B, CIN, H, W = 32, 128, 64, 64
COUT, KH, KW = 256, 3, 3
OH, OW = H - KH + 1, W - KW + 1  # 62, 62
N_CORES = 8
BL = B // N_CORES  # 4 images per core
NK = 4  # winograd components
P = OH // 2  # 31 row pairs
V_FREE = NK * P * W  # 4*31*64 = 7936
W_FREE = NK * KW * 2 * 128  # 3072
PAIRS_PER_BLOCK = 8  # 8*62 = 496 <= 512 (one PSUM bank per component)


def _conv_body(nc, tc, out_d, v_d, w_d):
    with (
        tc.tile_pool(name="const", bufs=1) as cpool,
        tc.tile_pool(name="vin", bufs=2) as v_pool,
        tc.tile_pool(name="psum", bufs=2, space=bass.MemorySpace.PSUM) as ps_pool,
        tc.tile_pool(name="stage", bufs=4) as st_pool,
        tc.tile_pool(name="tmp", bufs=4) as tmp_pool,
    ):
        w_sb = cpool.tile([128, W_FREE], BF16)
        wr = w_d.rearrange("p (x h co) -> p x h co", h=2, co=128)
        w_sv = w_sb.rearrange("p (x h co) -> p x h co", h=2, co=128)

        def dma_v(b, v_sb, chunked):
            vdr = v_d[b].rearrange("p (k pr x) -> p k pr x", k=NK, pr=P)
            vsv = v_sb.rearrange("p (k pr x) -> p k pr x", k=NK, pr=P)
            if chunked:
                # land pieces in exact consumption order of block 0: per
                # component k its h=0 weights + first pairs, then the rest
                nc.sync.dma_start(out=vsv[:, 0, 0:9, :], in_=vdr[:, 0, 0:9, :])
                for k in range(1, NK):
                    nc.sync.dma_start(
                        out=w_sv[:, 3 * k : 3 * k + 3, 0, :],
                        in_=wr[:, 3 * k : 3 * k + 3, 0, :],
                    )
                    nc.sync.dma_start(
                        out=vsv[:, k, 0:9, :], in_=vdr[:, k, 0:9, :]
                    )
                for r0, r1 in ((9, 17), (17, 25), (25, 31)):
                    nc.sync.dma_start(
                        out=vsv[:, :, r0:r1, :], in_=vdr[:, :, r0:r1, :]
                    )
            else:
                nc.sync.dma_start(out=v_sb, in_=v_d[b])

        # startup order: k=0/h=0 weights, then image-0 pieces, then the rest
        v_tiles = {}
        nc.sync.dma_start(out=w_sv[:, 0:3, 0, :], in_=wr[:, 0:3, 0, :])
        v_tiles[0] = v_pool.tile([128, V_FREE], BF16, tag="v", name="v_sb")
        dma_v(0, v_tiles[0], chunked=True)
        nc.sync.dma_start(out=w_sv[:, :, 1, :], in_=wr[:, :, 1, :])

        w_v = w_sb.rearrange("p (k kw h co) -> p k kw h co", k=NK, kw=KW, h=2)

        for b in range(BL):
            v_v = v_tiles[b].rearrange("p (k pr x) -> p k pr x", k=NK, pr=P)
            for h in range(2):
                for p0 in range(0, P, PAIRS_PER_BLOCK):
                    if h == 0 and p0 == 16 and b + 1 < BL:
                        # prefetch the next image once the current image's
                        # load has drained, well before it is needed
                        v_tiles[b + 1] = v_pool.tile(
                            [128, V_FREE], BF16, tag="v", name="v_sb"
                        )
                        dma_v(b + 1, v_tiles[b + 1], chunked=False)
                    npair = min(PAIRS_PER_BLOCK, P - p0)
                    nx = npair * OW
                    ps = ps_pool.tile([128, 4 * 512], F32, tag="ps")
                    for k in range(NK):
                        bank = ps[:, k * 512 : k * 512 + nx]
                        bank_v = bank.rearrange("p (pr x) -> p pr x", x=OW)
                        for kw in range(KW):
                            nc.tensor.matmul(
                                bank_v,
                                w_v[:, k, kw, h, :],
                                v_v[:, k, p0 : p0 + npair, kw : kw + OW],
                                start=(kw == 0),
                                stop=(kw == KW - 1),
                            )
                    st = st_pool.tile([128, 2 * PAIRS_PER_BLOCK * OW], BF16)
                    st_v = st.rearrange("p (r x) -> p r x", x=OW)
                    # Single PSUM reader: ACT bulk-copies all four banks to
                    # SBUF bf16 (PSUM readers serialize, so one big op frees
                    # the banks fastest); all combining then runs from SBUF.
                    sall = tmp_pool.tile(
                        [128, 4 * PAIRS_PER_BLOCK * OW], BF16, tag="sall"
                    )
                    sall_v = sall.rearrange("p (k c) -> p k c", k=NK)
                    nc.scalar.activation(
                        sall_v[:, :, :nx],
                        ps.rearrange("p (k c) -> p k c", k=NK)[:, :, :nx],
                        mybir.ActivationFunctionType.Copy,
                    )
                    # even rows: M0 + M1 + M2 via reduce over the comp axis
                    with nc.allow_low_precision("bf16 output staging"):
                        nc.vector.tensor_reduce(
                            st_v[:, 0 : 2 * npair : 2, :],
                            sall_v[:, :3, :nx].rearrange("p k c -> p c k"),
                            mybir.AxisListType.X,
                            mybir.AluOpType.add,
                        )
                    # odd rows: M1 - M2 - M3
                    t01 = tmp_pool.tile([128, PAIRS_PER_BLOCK * OW], BF16, tag="t01")
                    nc.vector.tensor_sub(
                        t01[:, :nx], sall_v[:, 1, :nx], sall_v[:, 2, :nx]
                    )
                    nc.gpsimd.tensor_sub(
                        st_v[:, 1 : 2 * npair : 2, :],
                        t01[:, :nx].rearrange("p (r x) -> p r x", x=OW),
                        sall_v[:, 3, :nx].rearrange("p (r x) -> p r x", x=OW),
                    )
                    nc.sync.dma_start(
                        out=out_d[b, h * 128 : (h + 1) * 128, 2 * p0 : 2 * (p0 + npair), :],
                        in_=st_v[:, : 2 * npair, :],
                    )


def build_module():
    nc = bacc.Bacc(
        "TRN2", target_bir_lowering=False, debug=False, num_devices=N_CORES
    )
    v_d = nc.dram_tensor("v_in", [BL, CIN, V_FREE], BF16, kind="ExternalInput").ap()
    w_d = nc.dram_tensor("w_t", [CIN, W_FREE], BF16, kind="ExternalInput").ap()
    out_d = nc.dram_tensor("out", [BL, COUT, OH, OW], BF16, kind="ExternalOutput").ap()
    with tile.TileContext(nc) as tc:
        _conv_body(nc, tc, out_d, v_d, w_d)
    nc.compile()
    return nc


_NC_CACHE = {}


def _get_module():
    if "nc" not in _NC_CACHE:
        _NC_CACHE["nc"] = build_module()
    return _NC_CACHE["nc"]


def _host_transforms(input_image: np.ndarray, weights: np.ndarray):
    bf16 = ml_dtypes.bfloat16
    d = input_image.astype(bf16).astype(np.float32)  # [B, C, 64, 64]
    V = np.empty((B, CIN, NK, P, W), np.float32)
    e0 = d[:, :, 0 : 2 * P : 2]  # rows 0,2,..,60
    e1 = d[:, :, 1 : 2 * P : 2]  # rows 1,3,..,61
    e2 = d[:, :, 2 : 2 * P + 2 : 2]  # rows 2,4,..,62
    e3 = d[:, :, 3 : 2 * P + 3 : 2]  # rows 3,5,..,63
    V[:, :, 0] = e0 - e2
    V[:, :, 1] = e1 + e2
    V[:, :, 2] = e2 - e1
    V[:, :, 3] = e1 - e3
    V = np.ascontiguousarray(V.reshape(B, CIN, V_FREE)).astype(bf16)

    G = np.array([[1, 0, 0], [0.5, 0.5, 0.5], [0.5, -0.5, 0.5], [0, 0, 1]], np.float32)
    # weights [co, ci, kh, kw] -> Wp[ci, k, kw, h, co']
    Wp = np.einsum("gk,ockw->cgwo", G, weights.astype(np.float32), optimize=True)
    Wp = Wp.reshape(CIN, NK, KW, 2, 128)
    Wp = np.ascontiguousarray(Wp.reshape(CIN, W_FREE)).astype(bf16)
    return V, Wp


def kernel(input_image: np.ndarray, weights: np.ndarray) -> np.ndarray:
    input_image = np.ascontiguousarray(input_image, dtype=np.float32)
    weights = np.ascontiguousarray(weights, dtype=np.float32)
    V, Wp = _host_transforms(input_image, weights)
    nc = _get_module()
    in_maps = [
        {"v_in": V[i * BL : (i + 1) * BL], "w_t": Wp} for i in range(N_CORES)
    ]
    res = run_bass_kernel_spmd(nc, in_maps, list(range(N_CORES))).results
    return np.concatenate(
        [r["out"].astype(np.float32) for r in res], axis=0
    )


def make_in_maps(input_image: np.ndarray, weights: np.ndarray):
    V, Wp = _host_transforms(
        np.ascontiguousarray(input_image, dtype=np.float32),
        np.ascontiguousarray(weights, dtype=np.float32),
    )
    return [{"v_in": V[i * BL : (i + 1) * BL], "w_t": Wp} for i in range(N_CORES)]


# revision 3
# speedup vs baseline: 1.1652x; 1.1652x over previous
"""Conv2d 3x3 VALID via 1D Winograd F(2,3) along H, batch-sharded on 8 cores.

Problem: input [32,128,64,64] f32, weights [256,128,3,3] f32 ->
output [32,256,62,62] f32 (stride 1, no padding).

Host-side (per kernel() call):
  - image cast f32 -> bf16, then the Winograd input transform along H:
    for row-pair p (31 pairs = 62 out rows), with d_i = image row 2p+i:
      V0 = d0 - d2, V1 = d1 + d2, V2 = d2 - d1, V3 = d1 - d3   (bf16)
  - weight transform G w over kh per kw and Cout half:
      W'0 = w0, W'1 = (w0+w1+w2)/2, W'2 = (w0-w1+w2)/2, W'3 = w2
    shipped pre-transposed as lhsT [ci, (k kw h co)] bf16.

Device-side (per core, 4 images):
  - M_k[co, p, x] = sum_kw sum_ci V_k[ci, p, x+kw] * W'_k,kw[ci, co]
    accumulated over kw into PSUM bank k (3 matmuls, N = npairs*62 <= 496).
  - per block of <=8 row pairs and Cout half: 12 matmuls into 4 banks,
    then the inverse transform:
      even out rows = M0 + M1 + M2   (DVE tensor_reduce over the 3 banks)
      odd  out rows = M1 - M2 - M3   (DVE sub + ACT negate-copy + GPSIMD add)
  - staging tile [co, 2*npairs, 62] f32 -> DMA to DRAM.
"""

import numpy as np
import ml_dtypes

import concourse.bass as bass
import concourse.mybir as mybir
import concourse.tile as tile
from concourse import bacc
from concourse.bass_utils import run_bass_kernel_spmd

F32 = mybir.dt.float32
BF16 = mybir.dt.bfloat16

B, CIN, H, W = 32, 128, 64, 64
COUT, KH, KW = 256, 3, 3
OH, OW = H - KH + 1, W - KW + 1  # 62, 62
N_CORES = 8
BL = B // N_CORES  # 4 images per core
NK = 4  # winograd components
P = OH // 2  # 31 row pairs
V_FREE = NK * P * W  # 4*31*64 = 7936
W_FREE = NK * KW * 2 * 128  # 3072
PAIRS_PER_BLOCK = 8  # 8*62 = 496 <= 512 (one PSUM bank per component)


def _conv_body(nc, tc, out_d, v_d, w_d):
    with (
        tc.tile_pool(name="const", bufs=1) as cpool,
        tc.tile_pool(name="vin", bufs=2) as v_pool,
        tc.tile_pool(name="psum", bufs=2, space=bass.MemorySpace.PSUM) as ps_pool,
        tc.tile_pool(name="stage", bufs=4) as st_pool,
        tc.tile_pool(name="tmp", bufs=4) as tmp_pool,
    ):
        w_sb = cpool.tile([128, W_FREE], BF16)
        wr = w_d.rearrange("p (x h co) -> p x h co", h=2, co=128)
        w_sv = w_sb.rearrange("p (x h co) -> p x h co", h=2, co=128)

        def dma_v(b, v_sb, chunked):
            vdr = v_d[b].rearrange("p (k pr x) -> p k pr x", k=NK, pr=P)
            vsv = v_sb.rearrange("p (k pr x) -> p k pr x", k=NK, pr=P)
            if chunked:
                # land pieces in exact consumption order of block 0: per
                # component k its h=0 weights + first pairs, then the rest
                nc.sync.dma_start(out=vsv[:, 0, 0:9, :], in_=vdr[:, 0, 0:9, :])
                for k in range(1, NK):
                    nc.sync.dma_start(
                        out=w_sv[:, 3 * k : 3 * k + 3, 0, :],
                        in_=wr[:, 3 * k : 3 * k + 3, 0, :],
                    )
                    nc.sync.dma_start(
                        out=vsv[:, k, 0:9, :], in_=vdr[:, k, 0:9, :]
                    )
                for r0, r1 in ((9, 17), (17, 25), (25, 31)):
                    nc.sync.dma_start(
                        out=vsv[:, :, r0:r1, :], in_=vdr[:, :, r0:r1, :]
                    )
            else:
                nc.sync.dma_start(out=v_sb, in_=v_d[b])

        # startup order: k=0/h=0 weights, then image-0 pieces, then the rest
        v_tiles = {}
        nc.sync.dma_start(out=w_sv[:, 0:3, 0, :], in_=wr[:, 0:3, 0, :])
        v_tiles[0] = v_pool.tile([128, V_FREE], BF16, tag="v", name="v_sb")
        dma_v(0, v_tiles[0], chunked=True)

        # Warm up the PE HAM clock gate during the initial DMA wait: ~4us of
        # dummy matmuls on uninitialized SBUF so the real stream starts at
        # full clock. Results land in a scratch PSUM bank, never read.
        scratch = cpool.tile([128, 128], BF16)
        nc.vector.memset(scratch, 0)
        ps_warm = ps_pool.tile([128, 4 * 512], F32, tag="ps", name="ps")
        for i in range(36):
            nc.tensor.matmul(
                ps_warm[:, :128],
                scratch,
                scratch,
                start=True,
                stop=True,
            )

        w_v = w_sb.rearrange("p (k kw h co) -> p k kw h co", k=NK, kw=KW, h=2)

        for b in range(BL):
            v_v = v_tiles[b].rearrange("p (k pr x) -> p k pr x", k=NK, pr=P)
            for h in range(2):
                for p0 in range(0, P, PAIRS_PER_BLOCK):
                    # image 0's load saturates inbound DMA until its h=0
                    # pass ends, so issue the h=1 weights and the first
                    # prefetch later than for steady-state images
                    if b == 0 and h == 0 and p0 == 24:
                        nc.sync.dma_start(
                            out=w_sv[:, :, 1, :], in_=wr[:, :, 1, :]
                        )
                    pf = (h == 1 and p0 == 0) if b == 0 else (h == 0 and p0 == 16)
                    if pf and b + 1 < BL:
                        v_tiles[b + 1] = v_pool.tile(
                            [128, V_FREE], BF16, tag="v", name="v_sb"
                        )
                        dma_v(b + 1, v_tiles[b + 1], chunked=False)
                    npair = min(PAIRS_PER_BLOCK, P - p0)
                    nx = npair * OW
                    ps = ps_pool.tile([128, 4 * 512], F32, tag="ps")
                    for k in range(NK):
                        bank = ps[:, k * 512 : k * 512 + nx]
                        bank_v = bank.rearrange("p (pr x) -> p pr x", x=OW)
                        for kw in range(KW):
                            nc.tensor.matmul(
                                bank_v,
                                w_v[:, k, kw, h, :],
                                v_v[:, k, p0 : p0 + npair, kw : kw + OW],
                                start=(kw == 0),
                                stop=(kw == KW - 1),
                            )
                    st = st_pool.tile([128, 2 * PAIRS_PER_BLOCK * OW], BF16)
                    st_v = st.rearrange("p (r x) -> p r x", x=OW)
                    # Single PSUM reader: ACT bulk-copies all four banks to
                    # SBUF bf16 (PSUM readers serialize, so one big op frees
                    # the banks fastest); all combining then runs from SBUF.
                    sall = tmp_pool.tile(
                        [128, 4 * PAIRS_PER_BLOCK * OW], BF16, tag="sall"
                    )
                    sall_v = sall.rearrange("p (k c) -> p k c", k=NK)
                    nc.scalar.activation(
                        sall_v[:, :, :nx],
                        ps.rearrange("p (k c) -> p k c", k=NK)[:, :, :nx],
                        mybir.ActivationFunctionType.Copy,
                    )
                    # even rows: M0 + M1 + M2 via reduce over the comp axis
                    with nc.allow_low_precision("bf16 output staging"):
                        nc.vector.tensor_reduce(
                            st_v[:, 0 : 2 * npair : 2, :],
                            sall_v[:, :3, :nx].rearrange("p k c -> p c k"),
                            mybir.AxisListType.X,
                            mybir.AluOpType.add,
                        )
                    # odd rows: M1 - M2 - M3
                    t01 = tmp_pool.tile([128, PAIRS_PER_BLOCK * OW], BF16, tag="t01")
                    nc.vector.tensor_sub(
                        t01[:, :nx], sall_v[:, 1, :nx], sall_v[:, 2, :nx]
                    )
                    nc.gpsimd.tensor_sub(
                        st_v[:, 1 : 2 * npair : 2, :],
                        t01[:, :nx].rearrange("p (r x) -> p r x", x=OW),
                        sall_v[:, 3, :nx].rearrange("p (r x) -> p r x", x=OW),
                    )
                    nc.sync.dma_start(
                        out=out_d[b, h * 128 : (h + 1) * 128, 2 * p0 : 2 * (p0 + npair), :],
                        in_=st_v[:, : 2 * npair, :],
                    )


def build_module():
    nc = bacc.Bacc(
        "TRN2", target_bir_lowering=False, debug=False, num_devices=N_CORES
    )
    v_d = nc.dram_tensor("v_in", [BL, CIN, V_FREE], BF16, kind="ExternalInput").ap()
    w_d = nc.dram_tensor("w_t", [CIN, W_FREE], BF16, kind="ExternalInput").ap()
    out_d = nc.dram_tensor("out", [BL, COUT, OH, OW], BF16, kind="ExternalOutput").ap()
    with tile.TileContext(nc) as tc:
        _conv_body(nc, tc, out_d, v_d, w_d)
    nc.compile()
    return nc


_NC_CACHE = {}


def _get_module():
    if "nc" not in _NC_CACHE:
        _NC_CACHE["nc"] = build_module()
    return _NC_CACHE["nc"]


def _host_transforms(input_image: np.ndarray, weights: np.ndarray):
    bf16 = ml_dtypes.bfloat16
    d = input_image.astype(bf16).astype(np.float32)  # [B, C, 64, 64]
    V = np.empty((B, CIN, NK, P, W), np.float32)
    e0 = d[:, :, 0 : 2 * P : 2]  # rows 0,2,..,60
    e1 = d[:, :, 1 : 2 * P : 2]  # rows 1,3,..,61
    e2 = d[:, :, 2 : 2 * P + 2 : 2]  # rows 2,4,..,62
    e3 = d[:, :, 3 : 2 * P + 3 : 2]  # rows 3,5,..,63
    V[:, :, 0] = e0 - e2
    V[:, :, 1] = e1 + e2
    V[:, :, 2] = e2 - e1
    V[:, :, 3] = e1 - e3
    V = np.ascontiguousarray(V.reshape(B, CIN, V_FREE)).astype(bf16)

    G = np.array([[1, 0, 0], [0.5, 0.5, 0.5], [0.5, -0.5, 0.5], [0, 0, 1]], np.float32)
    # weights [co, ci, kh, kw] -> Wp[ci, k, kw, h, co']
    Wp = np.einsum("gk,ockw->cgwo", G, weights.astype(np.float32), optimize=True)
    Wp = Wp.reshape(CIN, NK, KW, 2, 128)
    Wp = np.ascontiguousarray(Wp.reshape(CIN, W_FREE)).astype(bf16)
    return V, Wp


def kernel(input_image: np.ndarray, weights: np.ndarray) -> np.ndarray:
    input_image = np.ascontiguousarray(input_image, dtype=np.float32)
    weights = np.ascontiguousarray(weights, dtype=np.float32)
    V, Wp = _host_transforms(input_image, weights)
    nc = _get_module()
    in_maps = [
        {"v_in": V[i * BL : (i + 1) * BL], "w_t": Wp} for i in range(N_CORES)
    ]
    res = run_bass_kernel_spmd(nc, in_maps, list(range(N_CORES))).results
    return np.concatenate(
        [r["out"].astype(np.float32) for r in res], axis=0
    )


def make_in_maps(input_image: np.ndarray, weights: np.ndarray):
    V, Wp = _host_transforms(
        np.ascontiguousarray(input_image, dtype=np.float32),
        np.ascontiguousarray(weights, dtype=np.float32),
    )
    return [{"v_in": V[i * BL : (i + 1) * BL], "w_t": Wp} for i in range(N_CORES)]
